# revision 27
# baseline (speedup 1.0000x reference)
"""Trainium2 Bass kernel: KV-memory retrieval (pool -> cosine kNN -> softmax gather).

Strategy (8 cores): shard the 65536-slot memory across cores (8192 keys/values
each) and the 256-image batch across cores (32 each) for pooling + output.

Per core, single SPMD launch:
  1. pool its x shard -> qT [512, 32]; AllGather -> qT_all [512, 256]
  2. q norms via squares + ones-matmul over partitions -> rinv [b]
  3. stream key blocks: row-normalize (ACT square+accum, sqrt, DVE recip,
     DVE prescale to f32r), PE transpose-mode -> kT, f32r matmul1
     qT.T @ kT -> sim [256, 8192], per-block top-8 candidates (max8)
  4. local top-32 -> AllGather candidates -> global top-32 (sorted),
     threshold t, softmax stats gmax / Z (exp with per-partition scale/bias)
  5. dense w = exp(sim*rinv + bias) * (sim >= t)  (1/Z folded into bias),
     stored f32r
  6. matmul2 (f32r): values.T @ w -> partial matched.T [512, 256]
  7. transpose -> [256, 512], ReduceScatter(add) -> own batch shard [32, 512]
  8. broadcast over 784 spatial positions, DMA out [32, 512, 784]

f32r (single-pass fp32 on the PE, ~16-bit mantissa) is safe here: measured
sim noise ~1e-6 in cos units vs ~4e-4 gaps between rank 32/33.
Selection is done on raw r = q_sum . k_norm (scale-invariant per batch row);
1/||q|| enters only through the exp scale. Mean /784 cancels everywhere.
"""

import math

import numpy as np

import concourse.bacc as bacc
import concourse.mybir as mybir
import concourse.tile as tile
from concourse.bass import ts
from concourse.bass_utils import run_bass_kernel_spmd
from concourse.masks import make_identity

F32 = mybir.dt.float32
F32R = mybir.dt.float32r
AF = mybir.ActivationFunctionType
ALU = mybir.AluOpType

N_CORES = 8
NEG = -3.0e38


def build(B=256, C=512, HW=784, M=65536, K=32, n_cores=N_CORES, mb=512):
    """Build + bacc-compile the SPMD program. Returns nc."""
    BS = B // n_cores          # batches per core
    MS = M // n_cores          # memory slots per core
    CT = C // 128              # channel tiles (also contraction tiles)
    BT = B // 128 if B >= 128 else 1
    BTW = 128 if B >= 128 else B   # batch-tile width
    assert B % BTW == 0 and C % 128 == 0 and M % (n_cores * mb) == 0
    NMB = MS // mb             # key blocks per core
    KTPB = mb // 128           # 128-row key tiles per block
    R = math.ceil(K / 8)       # max8 rounds for exact top-K
    KPB = 8                    # top-8 per 512-block (validated sufficient)
    MT = MS // 128             # value tiles
    RG = [list(range(n_cores))]
    CC_AS = "Shared" if n_cores > 4 else "Local"

    nc = bacc.Bacc("TRN2", target_bir_lowering=False, debug=False,
                   num_devices=n_cores)

    xs = nc.dram_tensor("xs", [BS, C, HW], F32, kind="ExternalInput").ap()
    keys = nc.dram_tensor("keys", [MS, C], F32, kind="ExternalInput").ap()
    vals = nc.dram_tensor("vals", [MS, C], F32, kind="ExternalInput").ap()
    out = nc.dram_tensor("out", [BS, C, HW], F32, kind="ExternalOutput").ap()

    with tile.TileContext(nc) as tc:
        with (
            tc.tile_pool(name="consts", bufs=1) as consts,
            tc.tile_pool(name="persist", bufs=1) as persist,
            tc.tile_pool(name="dram", bufs=1, space="DRAM") as dram,
        ):
            identity = consts.tile([128, 128], F32)
            make_identity(nc, identity)
            identity_r = consts.tile([128, 128], F32R)
            nc.vector.tensor_copy(identity_r, identity)
            ones_col = consts.tile([128, 1], F32)
            nc.vector.memset(ones_col, 1.0)
            ones_hw = consts.tile([128, HW], F32)
            nc.vector.memset(ones_hw, 1.0)

            sim = [persist.tile([BTW, MS], F32, name=f"sim{i}")
                   for i in range(BT)]
            qTt = persist.tile([128, CT, B], F32, name="qTt")
            qT = [qTt[:, i] for i in range(CT)]
            qTr = persist.tile([128, CT, B], F32R, name="qTr")
            qTl = [persist.tile([128, BS], F32, name=f"qTl{i}")
                   for i in range(CT)]
            cand = [persist.tile([BTW, NMB * KPB], F32, name=f"cand{i}")
                    for i in range(BT)]
            rinv = [persist.tile([BTW, 1], F32, name=f"rinv{i}")
                    for i in range(BT)]
            bias2 = [persist.tile([BTW, 1], F32, name=f"bias2{i}")
                     for i in range(BT)]
            g32 = [persist.tile([BTW, R * 8], F32, name=f"g32{i}")
                   for i in range(BT)]
            mrow = [persist.tile([BTW, C], F32, name=f"mrow{i}")
                    for i in range(BT)]
            mTmy = [persist.tile([128, BS], F32, name=f"mTmy{i}")
                    for i in range(CT)]

            # ---------------- Phase P: pool x -> qT local ----------------
            with (
                tc.tile_pool(name="poolP", bufs=3) as pP,
            ):
                hw_a = 0
                for a in range(int(math.isqrt(HW)), 1, -1):
                    if HW % a == 0:
                        hw_a = a
                        break
                CTH = CT // 2
                for b in range(BS):
                    if b % 2 == 0:
                        xt2 = pP.tile([128, 2, CT, HW], F32, tag="xt2",
                                      bufs=3)
                        nc.sync.dma_start(
                            out=xt2,
                            in_=xs[b:b + 2].rearrange(
                                "b (ct p) hw -> p b ct hw", p=128))
                    xt = xt2[:, b % 2]
                    if hw_a > 1:
                        xp = pP.tile([128, CTH, HW // hw_a], F32, tag="xp")
                        nc.vector.tensor_reduce(
                            out=xp,
                            in_=xt[:, 0:CTH].rearrange(
                                "p ct (a b) -> p ct a b", a=HW // hw_a),
                            axis=mybir.AxisListType.X, op=ALU.add)
                        xq = pP.tile([128, CTH], F32, tag="xq")
                        nc.vector.tensor_reduce(
                            out=xq, in_=xp,
                            axis=mybir.AxisListType.X, op=ALU.add)
                    else:
                        xq = pP.tile([128, CTH], F32, tag="xq")
                        nc.vector.tensor_reduce(
                            out=xq, in_=xt[:, 0:CTH],
                            axis=mybir.AxisListType.X, op=ALU.add)
                    for ct in range(CTH):
                        nc.vector.tensor_copy(qTl[ct][:, b:b + 1],
                                              xq[:, ct:ct + 1])
                    for ct in range(CTH, CT):
                        xsc = pP.tile([128, HW], F32, tag="xsc")
                        nc.scalar.activation(
                            xsc, xt[:, ct], AF.Copy,
                            accum_out=qTl[ct][:, b:b + 1])


            # ---------------- Phase K: keys -> sim + block candidates -----
            PF = 7
            with (
                tc.tile_pool(name="poolK", bufs=2) as pK,
                tc.tile_pool(name="psumK", bufs=1, space="PSUM") as psK,
            ):
                kTbs = {}

                def key_prep(mbi):
                    ktb = pK.tile([128, KTPB, C], F32, tag="ktb", bufs=3)
                    nc.sync.dma_start(
                        out=ktb,
                        in_=keys[mbi * mb:(mbi + 1) * mb].rearrange(
                            "(kt p) c -> p kt c", p=128))
                    kno = pK.tile([128, KTPB, C], F32R, tag="kno", bufs=2)
                    for kt in range(KTPB):
                        ksq = pK.tile([128, C], F32, tag="ksq")
                        ssk = pK.tile([128, 1], F32, tag="ssk")
                        nc.scalar.activation(ksq, ktb[:, kt], AF.Square,
                                             accum_out=ssk)
                        kn = pK.tile([128, 1], F32, tag="kn")
                        nc.scalar.sqrt(kn, ssk)
                        rk = pK.tile([128, 1], F32, tag="rk")
                        nc.vector.reciprocal(rk, kn)
                        nc.vector.tensor_scalar_mul(kno[:, kt], ktb[:, kt],
                                                    rk)
                    pkt = [psK.tile([128, mb], F32R, tag=f"pkt{dt}",
                                    name=f"pkt{dt}") for dt in range(CT)]
                    for kt in range(KTPB):
                        for dt in range(CT):
                            nc.tensor.matmul(
                                pkt[dt][:, ts(kt, 128)],
                                lhsT=kno[:, kt, ts(dt, 128)],
                                rhs=identity_r, is_transpose=True,
                                start=True, stop=True, skip_group_check=True)
                    kTb = pK.tile([128, CT, mb], F32R, tag="kTb",
                                  bufs=PF + 2)
                    for dt in range(CT):
                        if dt % 2 == 0:
                            nc.vector.tensor_copy(kTb[:, dt], pkt[dt])
                        else:
                            nc.scalar.copy(kTb[:, dt], pkt[dt])
                    kTbs[mbi] = kTb

                # prefetch key prep for the first PF blocks; their DMAs
                # queue right behind pooling and fill the AllGather bubble,
                # and PE transposes run while the collective is in flight.
                for mbi in range(PF):
                    key_prep(mbi)

                # ---------------- AG1: gather queries ----------------
                qag_in = dram.tile([C, BS], F32)
                qag_out = dram.tile([n_cores, C, BS], F32, addr_space=CC_AS)
                for ct in range(CT):
                    nc.sync.dma_start(out=qag_in[ts(ct, 128), :],
                                      in_=qTl[ct])
                nc.gpsimd.collective_compute(
                    "AllGather", ALU.bypass, replica_groups=RG,
                    ins=[qag_in.opt()], outs=[qag_out.opt()])
                for r in range(n_cores):
                    nc.scalar.dma_start(
                        out=qTt[:, :, r * BS:(r + 1) * BS],
                        in_=qag_out[r].rearrange("(ct p) b -> p ct b",
                                                 p=128))
                nc.vector.tensor_copy(qTr, qTt)

                for mbi in range(NMB):
                    if mbi + PF < NMB:
                        key_prep(mbi + PF)
                    kTb = kTbs.pop(mbi)
                    for bt in range(BT):
                        psim = psK.tile([BTW, mb], F32, tag="psim", bufs=2)
                        for dt in range(CT):
                            nc.tensor.matmul(
                                psim, lhsT=qTr[:, dt, ts(bt, BTW)],
                                rhs=kTb[:, dt],
                                start=(dt == 0), stop=(dt == CT - 1),
                                skip_group_check=True)
                        sblk = sim[bt][:, ts(mbi, mb)]
                        if (mbi + bt) % 2 == 0:
                            nc.vector.tensor_copy(sblk, psim)
                        else:
                            nc.scalar.copy(sblk, psim)
                        nc.vector.max(
                            cand[bt][:, mbi * KPB:mbi * KPB + KPB], sblk)

            # ---------------- Phase Q: query norms ----------------
            with (
                tc.tile_pool(name="poolQ", bufs=2) as pQ,
                tc.tile_pool(name="psumQ", bufs=1, space="PSUM") as psQ,
            ):
                psum_ssq = psQ.tile([1, B], F32, tag="ssq")
                for ct in range(CT):
                    qsq = pQ.tile([128, B], F32, tag="qsq")
                    nc.scalar.square(qsq, qT[ct])
                    nc.tensor.matmul(psum_ssq, lhsT=ones_col, rhs=qsq,
                                     start=(ct == 0), stop=(ct == CT - 1))
                qn_row = pQ.tile([1, B], F32, tag="qn_row", bufs=1)
                nc.scalar.sqrt(qn_row, psum_ssq)
                ri_row = pQ.tile([1, B], F32, tag="ri_row", bufs=1)
                nc.vector.reciprocal(ri_row, qn_row)
                for bt in range(BT):
                    psum_rt = psQ.tile([BTW, 1], F32, tag="rt")
                    nc.tensor.matmul(
                        psum_rt, lhsT=ri_row[0:1, ts(bt, BTW)],
                        rhs=ones_col[0:1, 0:1], start=True, stop=True)
                    nc.vector.tensor_copy(rinv[bt], psum_rt)


            # ---------------- Phase G + W: top-K stats, dense matmul2 -----
            cd_in = dram.tile([B, K], F32)
            cd_out = dram.tile([n_cores, B, K], F32, addr_space=CC_AS)
            VB = 4                      # value tiles per DMA
            CW = 2                      # wexp chunk width (value tiles)
            with (
                tc.tile_pool(name="poolW", bufs=2) as pW,
                tc.tile_pool(name="psumW", bufs=1, space="PSUM") as psW,
                tc.tile_pool(name="poolG", bufs=1) as pG,
            ):
                vgroups = {}

                def issue_vals(g):
                    vtb = pW.tile([128, VB, C], F32, tag="vtb", bufs=2)
                    nc.sync.dma_start(
                        out=vtb,
                        in_=vals[g * VB * 128:(g + 1) * VB * 128].rearrange(
                            "(v p) c -> p v c", p=128))
                    vtr = pW.tile([128, VB, C], F32R, tag="vtr", bufs=2)
                    for v in range(VB):
                        if v % 2 == 0:
                            nc.vector.tensor_copy(vtr[:, v], vtb[:, v])
                        else:
                            nc.scalar.copy(vtr[:, v], vtb[:, v])
                    vgroups[g] = vtr

                # G1: local top-K per bt -> AllGather
                for bt in range(BT):
                    loc = pG.tile([BTW, R * 8], F32, tag="loc")
                    scr2 = pG.tile([BTW, NMB * KPB], F32, tag="scr2")
                    cur = cand[bt]
                    for r in range(R):
                        nc.vector.max(loc[:, r * 8:(r + 1) * 8], cur)
                        if r < R - 1:
                            nc.vector.match_replace(
                                scr2, in_to_replace=loc[:, r * 8:(r + 1) * 8],
                                in_values=cur, imm_value=NEG)
                            cur = scr2
                    nc.sync.dma_start(out=cd_in[ts(bt, BTW), :],
                                      in_=loc[:, 0:K])
                nc.gpsimd.collective_compute(
                    "AllGather", ALU.bypass, replica_groups=RG,
                    ins=[cd_in.opt()], outs=[cd_out.opt()])

                # value prefetch overlaps the collective + global merge
                issue_vals(0)
                issue_vals(1)

                # G2: global top-K + softmax stats
                for bt in range(BT):
                    gc = pG.tile([BTW, n_cores * K], F32, tag="gc")
                    nc.scalar.dma_start(
                        out=gc,
                        in_=cd_out[:, ts(bt, BTW), :].rearrange(
                            "r b k -> b r k"))
                    scr3 = pG.tile([BTW, n_cores * K], F32, tag="scr3")
                    cur = gc
                    for r in range(R):
                        nc.vector.max(g32[bt][:, r * 8:(r + 1) * 8], cur)
                        if r < R - 1:
                            nc.vector.match_replace(
                                scr3,
                                in_to_replace=g32[bt][:, r * 8:(r + 1) * 8],
                                in_values=cur, imm_value=NEG)
                            cur = scr3
                    # stats: nb = -gmax*rinv ; Z = sum exp((g - gmax)*rinv)
                    nb = pG.tile([BTW, 1], F32, tag="nb")
                    nc.vector.tensor_mul(nb, g32[bt][:, 0:1], rinv[bt])
                    nc.vector.tensor_scalar_mul(nb, nb, -1.0)
                    ex = pG.tile([BTW, K], F32, tag="ex")
                    zz = pG.tile([BTW, 1], F32, tag="zz")
                    nc.scalar.activation(ex, g32[bt][:, 0:K], AF.Exp,
                                         bias=nb, scale=rinv[bt],
                                         accum_out=zz)
                    lnz = pG.tile([BTW, 1], F32, tag="lnz")
                    nc.scalar.activation(lnz, zz, AF.Ln)
                    nc.vector.tensor_sub(bias2[bt], nb, lnz)

                # W: dense weights + matmul2
                pmB = [psW.tile([BTW, C], F32, tag=f"pmB{bt}",
                                name=f"pmB{bt}") for bt in range(BT)]
                for mt in range(MT):
                    g = mt // VB
                    if mt % VB == 0 and g not in vgroups:
                        issue_vals(g)
                    if mt % CW == 0:
                        we = [pW.tile([BTW, CW * 128], F32R, tag=f"we{bt}",
                                      bufs=2, name=f"we{bt}")
                              for bt in range(BT)]
                        for bt in range(BT):
                            schunk = sim[bt][:, mt * 128:(mt + CW) * 128]
                            nc.scalar.activation(we[bt], schunk, AF.Exp,
                                                 bias=bias2[bt],
                                                 scale=rinv[bt])
                            nc.vector.scalar_tensor_tensor(
                                out=we[bt], in0=schunk,
                                scalar=g32[bt][:, K - 1:K], in1=we[bt],
                                op0=ALU.is_ge, op1=ALU.mult)
                    vt = vgroups[g][:, mt % VB]
                    off = (mt % CW) * 128
                    pwt = psW.tile([128, B], F32R, tag="pwt", bufs=3)
                    for bt in range(BT):
                        nc.tensor.matmul(
                            pwt[:, ts(bt, BTW)],
                            lhsT=we[bt][:, off:off + 128],
                            rhs=identity_r[0:BTW, 0:BTW], is_transpose=True,
                            start=True, stop=True, skip_group_check=True)
                    wT = pW.tile([128, B], F32R, tag="wT", bufs=3)
                    if mt % 2 == 0:
                        nc.vector.tensor_copy(wT, pwt)
                    else:
                        nc.scalar.copy(wT, pwt)
                    for bt in range(BT):
                        nc.tensor.matmul(
                            pmB[bt], lhsT=wT[:, ts(bt, BTW)], rhs=vt,
                            start=(mt == 0), stop=(mt == MT - 1),
                            skip_group_check=True)
                for bt in range(BT):
                    nc.any.tensor_copy(mrow[bt], pmB[bt])

            # ---------------- Phase O: reduce-scatter + broadcast out -----
            mb_dram = dram.tile([B, C], F32)
            rs_out = dram.tile([BS, C], F32)
            with (
                tc.tile_pool(name="poolO", bufs=2) as pO,
                tc.tile_pool(name="psumO", bufs=1, space="PSUM") as psO,
            ):
                for bt in range(BT):
                    nc.sync.dma_start(out=mb_dram[ts(bt, BTW), :],
                                      in_=mrow[bt])
                nc.gpsimd.collective_compute(
                    "ReduceScatter", ALU.add, replica_groups=RG,
                    ins=[mb_dram.opt()], outs=[rs_out.opt()])
                mmy = pO.tile([BS, C], F32, tag="mmy", bufs=1)
                nc.scalar.dma_start(out=mmy, in_=rs_out)
                for dt in range(CT):
                    pmt = psO.tile([128, BS], F32, tag="pmt", bufs=2)
                    nc.tensor.matmul(
                        pmt, lhsT=mmy[:, ts(dt, 128)],
                        rhs=identity[0:BS, 0:BS], is_transpose=True,
                        start=True, stop=True, skip_group_check=True)
                    nc.any.tensor_copy(mTmy[dt], pmt)
                for b2 in range(BS // 2):
                    ot = pO.tile([128, 2, CT, HW], F32, tag="ot", bufs=3)
                    for bb in range(2):
                        b = 2 * b2 + bb
                        for dt in range(CT):
                            col = mTmy[dt][:, b:b + 1]
                            if (dt + bb) % 2 == 0:
                                nc.vector.tensor_scalar_mul(
                                    ot[:, bb, dt], ones_hw, col)
                            else:
                                nc.scalar.mul(ot[:, bb, dt], ones_hw, col)
                    nc.sync.dma_start(
                        out=out[2 * b2:2 * b2 + 2].rearrange(
                            "b (ct p) hw -> p b ct hw", p=128),
                        in_=ot)

    nc.compile()
    return nc


_CACHE = {}
TRACE = False
TRACE_DIR = None
LAST_RESULT = None


def _get(shape_key):
    if shape_key not in _CACHE:
        _CACHE[shape_key] = build(*shape_key)
    return _CACHE[shape_key]


def kernel(x, keys, values, topk, **_ignored):
    K = int(np.asarray(topk))
    B, C, H, W = x.shape
    M, D = keys.shape
    HW = H * W
    nc = _get((B, C, HW, M, K, N_CORES))
    BS, MS = B // N_CORES, M // N_CORES
    x3 = np.ascontiguousarray(x.reshape(B, C, HW)).astype(np.float32,
                                                          copy=False)
    keys = np.ascontiguousarray(keys).astype(np.float32, copy=False)
    values = np.ascontiguousarray(values).astype(np.float32, copy=False)
    in_maps = [{
        "xs": x3[c * BS:(c + 1) * BS],
        "keys": keys[c * MS:(c + 1) * MS],
        "vals": values[c * MS:(c + 1) * MS],
    } for c in range(N_CORES)]
    global LAST_RESULT
    res = run_bass_kernel_spmd(nc, in_maps, core_ids=list(range(N_CORES)),
                               trace=TRACE, tmpdir=TRACE_DIR)
    LAST_RESULT = res
    outs = [res.results[c]["out"] for c in range(N_CORES)]
    return np.concatenate(outs, axis=0).reshape(B, C, H, W)


# revision 28
# speedup vs baseline: 1.0050x; 1.0050x over previous
"""Trainium2 Bass kernel: KV-memory retrieval (pool -> cosine kNN -> softmax gather).

Strategy (8 cores): shard the 65536-slot memory across cores (8192 keys/values
each) and the 256-image batch across cores (32 each) for pooling + output.

Per core, single SPMD launch:
  1. pool its x shard -> qT [512, 32]; AllGather -> qT_all [512, 256]
  2. q norms via squares + ones-matmul over partitions -> rinv [b]
  3. stream key blocks: row-normalize (ACT square+accum, sqrt, DVE recip,
     DVE prescale to f32r), PE transpose-mode -> kT, f32r matmul1
     qT.T @ kT -> sim [256, 8192], per-block top-8 candidates (max8)
  4. local top-32 -> AllGather candidates -> global top-32 (sorted),
     threshold t, softmax stats gmax / Z (exp with per-partition scale/bias)
  5. dense w = exp(sim*rinv + bias) * (sim >= t)  (1/Z folded into bias),
     stored f32r
  6. matmul2 (f32r): values.T @ w -> partial matched.T [512, 256]
  7. transpose -> [256, 512], ReduceScatter(add) -> own batch shard [32, 512]
  8. broadcast over 784 spatial positions, DMA out [32, 512, 784]

f32r (single-pass fp32 on the PE, ~16-bit mantissa) is safe here: measured
sim noise ~1e-6 in cos units vs ~4e-4 gaps between rank 32/33.
Selection is done on raw r = q_sum . k_norm (scale-invariant per batch row);
1/||q|| enters only through the exp scale. Mean /784 cancels everywhere.
"""

import math

import numpy as np

import concourse.bacc as bacc
import concourse.mybir as mybir
import concourse.tile as tile
from concourse.bass import ts
from concourse.bass_utils import run_bass_kernel_spmd
from concourse.masks import make_identity

F32 = mybir.dt.float32
F32R = mybir.dt.float32r
AF = mybir.ActivationFunctionType
ALU = mybir.AluOpType

N_CORES = 8
NEG = -3.0e38


def build(B=256, C=512, HW=784, M=65536, K=32, n_cores=N_CORES, mb=512):
    """Build + bacc-compile the SPMD program. Returns nc."""
    BS = B // n_cores          # batches per core
    MS = M // n_cores          # memory slots per core
    CT = C // 128              # channel tiles (also contraction tiles)
    BT = B // 128 if B >= 128 else 1
    BTW = 128 if B >= 128 else B   # batch-tile width
    assert B % BTW == 0 and C % 128 == 0 and M % (n_cores * mb) == 0
    NMB = MS // mb             # key blocks per core
    KTPB = mb // 128           # 128-row key tiles per block
    R = math.ceil(K / 8)       # max8 rounds for exact top-K
    KPB = 8                    # top-8 per 512-block (validated sufficient)
    MT = MS // 128             # value tiles
    RG = [list(range(n_cores))]
    CC_AS = "Shared" if n_cores > 4 else "Local"

    nc = bacc.Bacc("TRN2", target_bir_lowering=False, debug=False,
                   num_devices=n_cores)

    xs = nc.dram_tensor("xs", [BS, C, HW], F32, kind="ExternalInput").ap()
    keys = nc.dram_tensor("keys", [MS, C], F32, kind="ExternalInput").ap()
    vals = nc.dram_tensor("vals", [MS, C], F32, kind="ExternalInput").ap()
    out = nc.dram_tensor("out", [BS, C, HW], F32, kind="ExternalOutput").ap()

    with tile.TileContext(nc) as tc:
        with (
            tc.tile_pool(name="consts", bufs=1) as consts,
            tc.tile_pool(name="persist", bufs=1) as persist,
            tc.tile_pool(name="dram", bufs=1, space="DRAM") as dram,
        ):
            identity = consts.tile([128, 128], F32)
            make_identity(nc, identity)
            identity_r = consts.tile([128, 128], F32R)
            nc.vector.tensor_copy(identity_r, identity)
            ones_col = consts.tile([128, 1], F32)
            nc.vector.memset(ones_col, 1.0)
            ones_hw = consts.tile([128, HW], F32)
            nc.vector.memset(ones_hw, 1.0)

            sim = [persist.tile([BTW, MS], F32, name=f"sim{i}")
                   for i in range(BT)]
            qTt = persist.tile([128, CT, B], F32, name="qTt")
            qT = [qTt[:, i] for i in range(CT)]
            qTr = persist.tile([128, CT, B], F32R, name="qTr")
            qTl = [persist.tile([128, BS], F32, name=f"qTl{i}")
                   for i in range(CT)]
            cand = [persist.tile([BTW, NMB * KPB], F32, name=f"cand{i}")
                    for i in range(BT)]
            rinv = [persist.tile([BTW, 1], F32, name=f"rinv{i}")
                    for i in range(BT)]
            bias2 = [persist.tile([BTW, 1], F32, name=f"bias2{i}")
                     for i in range(BT)]
            g32 = [persist.tile([BTW, R * 8], F32, name=f"g32{i}")
                   for i in range(BT)]
            mrow = [persist.tile([BTW, C], F32, name=f"mrow{i}")
                    for i in range(BT)]
            mTmy = [persist.tile([128, BS], F32, name=f"mTmy{i}")
                    for i in range(CT)]

            # ---------------- Phase P: pool x -> qT local ----------------
            with (
                tc.tile_pool(name="poolP", bufs=3) as pP,
            ):
                hw_a = 0
                for a in range(int(math.isqrt(HW)), 1, -1):
                    if HW % a == 0:
                        hw_a = a
                        break
                CTH = CT // 2
                for b in range(BS):
                    if b % 2 == 0:
                        xt2 = pP.tile([128, 2, CT, HW], F32, tag="xt2",
                                      bufs=3)
                        nc.sync.dma_start(
                            out=xt2,
                            in_=xs[b:b + 2].rearrange(
                                "b (ct p) hw -> p b ct hw", p=128))
                    xt = xt2[:, b % 2]
                    if hw_a > 1:
                        xp = pP.tile([128, CTH, HW // hw_a], F32, tag="xp")
                        nc.vector.tensor_reduce(
                            out=xp,
                            in_=xt[:, 0:CTH].rearrange(
                                "p ct (a b) -> p ct a b", a=HW // hw_a),
                            axis=mybir.AxisListType.X, op=ALU.add)
                        xq = pP.tile([128, CTH], F32, tag="xq")
                        nc.vector.tensor_reduce(
                            out=xq, in_=xp,
                            axis=mybir.AxisListType.X, op=ALU.add)
                    else:
                        xq = pP.tile([128, CTH], F32, tag="xq")
                        nc.vector.tensor_reduce(
                            out=xq, in_=xt[:, 0:CTH],
                            axis=mybir.AxisListType.X, op=ALU.add)
                    for ct in range(CTH):
                        nc.vector.tensor_copy(qTl[ct][:, b:b + 1],
                                              xq[:, ct:ct + 1])
                    for ct in range(CTH, CT):
                        xsc = pP.tile([128, HW], F32, tag="xsc")
                        nc.scalar.activation(
                            xsc, xt[:, ct], AF.Copy,
                            accum_out=qTl[ct][:, b:b + 1])


            # ---------------- Phase K: keys -> sim + block candidates -----
            PF = 7
            with (
                tc.tile_pool(name="poolK", bufs=2) as pK,
                tc.tile_pool(name="psumK", bufs=1, space="PSUM") as psK,
            ):
                kTbs = {}

                def key_prep(mbi):
                    ktb = pK.tile([128, KTPB, C], F32, tag="ktb", bufs=3)
                    nc.sync.dma_start(
                        out=ktb,
                        in_=keys[mbi * mb:(mbi + 1) * mb].rearrange(
                            "(kt p) c -> p kt c", p=128))
                    kno = pK.tile([128, KTPB, C], F32R, tag="kno", bufs=2)
                    for kt in range(KTPB):
                        ksq = pK.tile([128, C], F32, tag="ksq")
                        ssk = pK.tile([128, 1], F32, tag="ssk")
                        nc.scalar.activation(ksq, ktb[:, kt], AF.Square,
                                             accum_out=ssk)
                        kn = pK.tile([128, 1], F32, tag="kn")
                        nc.scalar.sqrt(kn, ssk)
                        rk = pK.tile([128, 1], F32, tag="rk")
                        nc.vector.reciprocal(rk, kn)
                        nc.vector.tensor_scalar_mul(kno[:, kt], ktb[:, kt],
                                                    rk)
                    pkt = [psK.tile([128, mb], F32R, tag=f"pkt{dt}",
                                    name=f"pkt{dt}") for dt in range(CT)]
                    for kt in range(KTPB):
                        for dt in range(CT):
                            nc.tensor.matmul(
                                pkt[dt][:, ts(kt, 128)],
                                lhsT=kno[:, kt, ts(dt, 128)],
                                rhs=identity_r, is_transpose=True,
                                start=True, stop=True, skip_group_check=True)
                    kTb = pK.tile([128, CT, mb], F32R, tag="kTb",
                                  bufs=PF + 2)
                    for dt in range(CT):
                        if dt % 2 == 0:
                            nc.vector.tensor_copy(kTb[:, dt], pkt[dt])
                        else:
                            nc.scalar.copy(kTb[:, dt], pkt[dt])
                    kTbs[mbi] = kTb

                # ---------------- AG1: gather queries ----------------
                qag_in = dram.tile([C, BS], F32)
                qag_out = dram.tile([n_cores, C, BS], F32, addr_space=CC_AS)
                for ct in range(CT):
                    nc.sync.dma_start(out=qag_in[ts(ct, 128), :],
                                      in_=qTl[ct])
                nc.gpsimd.collective_compute(
                    "AllGather", ALU.bypass, replica_groups=RG,
                    ins=[qag_in.opt()], outs=[qag_out.opt()])

                # prefetch key prep for the first PF blocks; their DMAs
                # fill the AllGather window and PE transposes run while
                # the collective is in flight.
                for mbi in range(PF):
                    key_prep(mbi)

                for r in range(n_cores):
                    nc.scalar.dma_start(
                        out=qTt[:, :, r * BS:(r + 1) * BS],
                        in_=qag_out[r].rearrange("(ct p) b -> p ct b",
                                                 p=128))
                nc.vector.tensor_copy(qTr, qTt)

                for mbi in range(NMB):
                    if mbi + PF < NMB:
                        key_prep(mbi + PF)
                    kTb = kTbs.pop(mbi)
                    for bt in range(BT):
                        psim = psK.tile([BTW, mb], F32, tag="psim", bufs=2)
                        for dt in range(CT):
                            nc.tensor.matmul(
                                psim, lhsT=qTr[:, dt, ts(bt, BTW)],
                                rhs=kTb[:, dt],
                                start=(dt == 0), stop=(dt == CT - 1),
                                skip_group_check=True)
                        sblk = sim[bt][:, ts(mbi, mb)]
                        if (mbi + bt) % 2 == 0:
                            nc.vector.tensor_copy(sblk, psim)
                        else:
                            nc.scalar.copy(sblk, psim)
                        nc.vector.max(
                            cand[bt][:, mbi * KPB:mbi * KPB + KPB], sblk)

            # ---------------- Phase Q: query norms ----------------
            with (
                tc.tile_pool(name="poolQ", bufs=2) as pQ,
                tc.tile_pool(name="psumQ", bufs=1, space="PSUM") as psQ,
            ):
                psum_ssq = psQ.tile([1, B], F32, tag="ssq")
                for ct in range(CT):
                    qsq = pQ.tile([128, B], F32, tag="qsq")
                    nc.scalar.square(qsq, qT[ct])
                    nc.tensor.matmul(psum_ssq, lhsT=ones_col, rhs=qsq,
                                     start=(ct == 0), stop=(ct == CT - 1))
                qn_row = pQ.tile([1, B], F32, tag="qn_row", bufs=1)
                nc.scalar.sqrt(qn_row, psum_ssq)
                ri_row = pQ.tile([1, B], F32, tag="ri_row", bufs=1)
                nc.vector.reciprocal(ri_row, qn_row)
                for bt in range(BT):
                    psum_rt = psQ.tile([BTW, 1], F32, tag="rt")
                    nc.tensor.matmul(
                        psum_rt, lhsT=ri_row[0:1, ts(bt, BTW)],
                        rhs=ones_col[0:1, 0:1], start=True, stop=True)
                    nc.vector.tensor_copy(rinv[bt], psum_rt)


            # ---------------- Phase G + W: top-K stats, dense matmul2 -----
            cd_in = dram.tile([B, K], F32)
            cd_out = dram.tile([n_cores, B, K], F32, addr_space=CC_AS)
            VB = 4                      # value tiles per DMA
            CW = 2                      # wexp chunk width (value tiles)
            with (
                tc.tile_pool(name="poolW", bufs=2) as pW,
                tc.tile_pool(name="psumW", bufs=1, space="PSUM") as psW,
                tc.tile_pool(name="poolG", bufs=1) as pG,
            ):
                vgroups = {}

                def issue_vals(g):
                    vtb = pW.tile([128, VB, C], F32, tag="vtb", bufs=2)
                    nc.sync.dma_start(
                        out=vtb,
                        in_=vals[g * VB * 128:(g + 1) * VB * 128].rearrange(
                            "(v p) c -> p v c", p=128))
                    vtr = pW.tile([128, VB, C], F32R, tag="vtr", bufs=2)
                    for v in range(VB):
                        if v % 2 == 0:
                            nc.vector.tensor_copy(vtr[:, v], vtb[:, v])
                        else:
                            nc.scalar.copy(vtr[:, v], vtb[:, v])
                    vgroups[g] = vtr

                # G1: local top-K per bt -> AllGather
                for bt in range(BT):
                    loc = pG.tile([BTW, R * 8], F32, tag="loc")
                    scr2 = pG.tile([BTW, NMB * KPB], F32, tag="scr2")
                    cur = cand[bt]
                    for r in range(R):
                        nc.vector.max(loc[:, r * 8:(r + 1) * 8], cur)
                        if r < R - 1:
                            nc.vector.match_replace(
                                scr2, in_to_replace=loc[:, r * 8:(r + 1) * 8],
                                in_values=cur, imm_value=NEG)
                            cur = scr2
                    nc.sync.dma_start(out=cd_in[ts(bt, BTW), :],
                                      in_=loc[:, 0:K])
                nc.gpsimd.collective_compute(
                    "AllGather", ALU.bypass, replica_groups=RG,
                    ins=[cd_in.opt()], outs=[cd_out.opt()])

                # value prefetch overlaps the collective + global merge
                issue_vals(0)
                issue_vals(1)

                # G2: global top-K + softmax stats
                for bt in range(BT):
                    gc = pG.tile([BTW, n_cores * K], F32, tag="gc")
                    nc.scalar.dma_start(
                        out=gc,
                        in_=cd_out[:, ts(bt, BTW), :].rearrange(
                            "r b k -> b r k"))
                    scr3 = pG.tile([BTW, n_cores * K], F32, tag="scr3")
                    cur = gc
                    for r in range(R):
                        nc.vector.max(g32[bt][:, r * 8:(r + 1) * 8], cur)
                        if r < R - 1:
                            nc.vector.match_replace(
                                scr3,
                                in_to_replace=g32[bt][:, r * 8:(r + 1) * 8],
                                in_values=cur, imm_value=NEG)
                            cur = scr3
                    # stats: nb = -gmax*rinv ; Z = sum exp((g - gmax)*rinv)
                    nb = pG.tile([BTW, 1], F32, tag="nb")
                    nc.vector.tensor_mul(nb, g32[bt][:, 0:1], rinv[bt])
                    nc.vector.tensor_scalar_mul(nb, nb, -1.0)
                    ex = pG.tile([BTW, K], F32, tag="ex")
                    zz = pG.tile([BTW, 1], F32, tag="zz")
                    nc.scalar.activation(ex, g32[bt][:, 0:K], AF.Exp,
                                         bias=nb, scale=rinv[bt],
                                         accum_out=zz)
                    lnz = pG.tile([BTW, 1], F32, tag="lnz")
                    nc.scalar.activation(lnz, zz, AF.Ln)
                    nc.vector.tensor_sub(bias2[bt], nb, lnz)

                # W: dense weights + matmul2
                pmB = [psW.tile([BTW, C], F32, tag=f"pmB{bt}",
                                name=f"pmB{bt}") for bt in range(BT)]
                for mt in range(MT):
                    g = mt // VB
                    if mt % VB == 0 and g not in vgroups:
                        issue_vals(g)
                    if mt % CW == 0:
                        we = [pW.tile([BTW, CW * 128], F32R, tag=f"we{bt}",
                                      bufs=2, name=f"we{bt}")
                              for bt in range(BT)]
                        for bt in range(BT):
                            schunk = sim[bt][:, mt * 128:(mt + CW) * 128]
                            nc.scalar.activation(we[bt], schunk, AF.Exp,
                                                 bias=bias2[bt],
                                                 scale=rinv[bt])
                            nc.vector.scalar_tensor_tensor(
                                out=we[bt], in0=schunk,
                                scalar=g32[bt][:, K - 1:K], in1=we[bt],
                                op0=ALU.is_ge, op1=ALU.mult)
                    vt = vgroups[g][:, mt % VB]
                    off = (mt % CW) * 128
                    pwt = psW.tile([128, B], F32R, tag="pwt", bufs=3)
                    for bt in range(BT):
                        nc.tensor.matmul(
                            pwt[:, ts(bt, BTW)],
                            lhsT=we[bt][:, off:off + 128],
                            rhs=identity_r[0:BTW, 0:BTW], is_transpose=True,
                            start=True, stop=True, skip_group_check=True)
                    wT = pW.tile([128, B], F32R, tag="wT", bufs=3)
                    if mt % 2 == 0:
                        nc.vector.tensor_copy(wT, pwt)
                    else:
                        nc.scalar.copy(wT, pwt)
                    for bt in range(BT):
                        nc.tensor.matmul(
                            pmB[bt], lhsT=wT[:, ts(bt, BTW)], rhs=vt,
                            start=(mt == 0), stop=(mt == MT - 1),
                            skip_group_check=True)
                for bt in range(BT):
                    nc.any.tensor_copy(mrow[bt], pmB[bt])

            # ---------------- Phase O: reduce-scatter + broadcast out -----
            mb_dram = dram.tile([B, C], F32)
            rs_out = dram.tile([BS, C], F32)
            with (
                tc.tile_pool(name="poolO", bufs=2) as pO,
                tc.tile_pool(name="psumO", bufs=1, space="PSUM") as psO,
            ):
                for bt in range(BT):
                    nc.sync.dma_start(out=mb_dram[ts(bt, BTW), :],
                                      in_=mrow[bt])
                nc.gpsimd.collective_compute(
                    "ReduceScatter", ALU.add, replica_groups=RG,
                    ins=[mb_dram.opt()], outs=[rs_out.opt()])
                mmy = pO.tile([BS, C], F32, tag="mmy", bufs=1)
                nc.scalar.dma_start(out=mmy, in_=rs_out)
                for dt in range(CT):
                    pmt = psO.tile([128, BS], F32, tag="pmt", bufs=2)
                    nc.tensor.matmul(
                        pmt, lhsT=mmy[:, ts(dt, 128)],
                        rhs=identity[0:BS, 0:BS], is_transpose=True,
                        start=True, stop=True, skip_group_check=True)
                    nc.any.tensor_copy(mTmy[dt], pmt)
                for b2 in range(BS // 2):
                    ot = pO.tile([128, 2, CT, HW], F32, tag="ot", bufs=3)
                    for bb in range(2):
                        b = 2 * b2 + bb
                        for dt in range(CT):
                            col = mTmy[dt][:, b:b + 1]
                            if (dt + bb) % 2 == 0:
                                nc.vector.tensor_scalar_mul(
                                    ot[:, bb, dt], ones_hw, col)
                            else:
                                nc.scalar.mul(ot[:, bb, dt], ones_hw, col)
                    nc.sync.dma_start(
                        out=out[2 * b2:2 * b2 + 2].rearrange(
                            "b (ct p) hw -> p b ct hw", p=128),
                        in_=ot)

    nc.compile()
    return nc


_CACHE = {}
TRACE = False
TRACE_DIR = None
LAST_RESULT = None


def _get(shape_key):
    if shape_key not in _CACHE:
        _CACHE[shape_key] = build(*shape_key)
    return _CACHE[shape_key]


def kernel(x, keys, values, topk, **_ignored):
    K = int(np.asarray(topk))
    B, C, H, W = x.shape
    M, D = keys.shape
    HW = H * W
    nc = _get((B, C, HW, M, K, N_CORES))
    BS, MS = B // N_CORES, M // N_CORES
    x3 = np.ascontiguousarray(x.reshape(B, C, HW)).astype(np.float32,
                                                          copy=False)
    keys = np.ascontiguousarray(keys).astype(np.float32, copy=False)
    values = np.ascontiguousarray(values).astype(np.float32, copy=False)
    in_maps = [{
        "xs": x3[c * BS:(c + 1) * BS],
        "keys": keys[c * MS:(c + 1) * MS],
        "vals": values[c * MS:(c + 1) * MS],
    } for c in range(N_CORES)]
    global LAST_RESULT
    res = run_bass_kernel_spmd(nc, in_maps, core_ids=list(range(N_CORES)),
                               trace=TRACE, tmpdir=TRACE_DIR)
    LAST_RESULT = res
    outs = [res.results[c]["out"] for c in range(N_CORES)]
    return np.concatenate(outs, axis=0).reshape(B, C, H, W)
